# revision 3
# baseline (speedup 1.0000x reference)
"""CBOW negative-sampling loss kernel for Trainium2 (8 NeuronCores).

Problem (see reference):
    context_embeds = in_W[context].mean(axis=1)          # [B, D]
    true_embeds    = out_W[center.squeeze(1)]            # [B, D]
    pos_loss = softplus(-sum(context_embeds*true_embeds, -1)).mean()
    neg_embeds = out_W[neg_context]                      # [B, K, D]
    neg_loss = softplus(einsum('bkd,bd->bk', ...)).sum(-1).mean()
    out = pos_loss + neg_loss                            # scalar

Strategy: data-parallel over batch across 8 cores (2048 rows each);
embedding tables replicated per core.  Each core gathers its rows with
SWDGE indirect DMA (one 512B descriptor per embedding row), computes
dot products + softplus on DVE/ACT, and reduces to one partial-sum
scalar.  Host sums the 8 partials and divides by B.

Row layout per core: batch row b = chunk*128 + p lives on partition p,
chunk index c in the free dim (16 chunks of 128 rows).  Gathers are
issued per "super-chunk" of SC=4 chunks so DMA overlaps compute.

The walrus build in this container encodes at most ONE semaphore wait
per instruction ("Too many sync wait commands") and rejects the raw-ISA
InstTensorTensorReduce ("ISA wrong length"), so: waits are split onto
single-wait NoOps at Tile lowering time (PatchedTileContext below), and
dots use tensor_tensor + tensor_reduce instead.
"""

import numpy as np

VOCAB = 100000
DIM = 128
BATCH = 16384
CTX = 8
K_NEG = 10
N_CORES = 8
P = 128

B_CORE = BATCH // N_CORES          # 2048
N_CHUNKS = B_CORE // P             # 16
SC = 4                             # chunks per gather super-chunk
W_COLS = 1 + K_NEG                 # center + negatives share the out_W gather

_CACHE = {}


def _patched_tile_context():
    import concourse.mybir as mybir
    import concourse.tile as tile
    from concourse.vector_clock import ScopedClock

    class PatchedTileContext(tile.TileContext):
        """Split multi-wait sync_infos: this container's walrus codegen
        accepts only one semaphore wait (and update) per instruction."""

        def _add_instruction(self, inst):
            si = getattr(inst, "sync_info", None)
            if si is not None and len(si.on_wait) > 1:
                waits = list(si.on_wait)
                for w in waits[:-1]:
                    nop = mybir.InstNoOp(
                        name=f"I-{self.nc.next_id()}-waitsplit",
                        engine=inst.engine,
                        sync_info=mybir.SyncInfo(on_wait=[w], on_update=[]),
                        bass_nofuse=True,
                    )
                    super()._add_instruction(nop)
                inst.sync_info = mybir.SyncInfo(
                    on_wait=[waits[-1]], on_update=list(si.on_update)
                )
            super()._add_instruction(inst)

        def _drain_and_barrier(self, tick_clock, wait_clock):
            drain_inst = self.nc.sync.drain()
            wait_clock.add_sem_waits(
                drain_inst.ins, ScopedClock({None: tick_clock.global_clock})
            )
            si = drain_inst.ins.sync_info
            if si is not None and len(si.on_wait) > 1:
                waits = list(si.on_wait)
                ups = list(si.on_update)
                drain_inst.ins.sync_info = mybir.SyncInfo(
                    on_wait=waits[:1], on_update=[]
                )
                for i, w in enumerate(waits[1:]):
                    d2 = self.nc.sync.drain()
                    last = i == len(waits) - 2
                    d2.ins.sync_info = mybir.SyncInfo(
                        on_wait=[w], on_update=ups if last else []
                    )
            self.nc.all_engine_barrier()
            popped = self.nc._tile_sem_poison_stack.pop()
            assert popped is self._sem_poison
            self.nc.clear_and_free_semaphores(list(self.sems.allocated().values()))
            self.nc.all_engine_barrier()

    return PatchedTileContext


def build_bass(vocab=VOCAB, n_chunks=N_CHUNKS, sc=SC):
    """Build the per-core Bass program."""
    import concourse.bass as bass
    import concourse.mybir as mybir

    f32 = mybir.dt.float32
    i32 = mybir.dt.int32
    n_sc = n_chunks // sc
    TileContext = _patched_tile_context()

    nc = bass.Bass()

    ctx_idx_d = nc.dram_tensor("ctx_idx", [P, n_chunks * CTX], i32, kind="ExternalInput")
    w_idx_d = nc.dram_tensor("w_idx", [P, n_chunks * W_COLS], i32, kind="ExternalInput")
    in_w_d = nc.dram_tensor("in_w", [vocab, DIM], f32, kind="ExternalInput")
    out_w_d = nc.dram_tensor("out_w", [vocab, DIM], f32, kind="ExternalInput")
    loss_d = nc.dram_tensor("loss", [1, 1], f32, kind="ExternalOutput")

    with TileContext(nc) as tc:
        with (
            tc.tile_pool(name="idx", bufs=1) as ipool,
            tc.tile_pool(name="gather", bufs=2) as gpool,
            tc.tile_pool(name="work", bufs=2) as wpool,
            tc.tile_pool(name="accp", bufs=1) as apool,
            tc.tile_pool(name="psum", bufs=1, space="PSUM") as ppool,
        ):
            ctx_idx = ipool.tile([P, n_chunks * CTX], i32)
            w_idx = ipool.tile([P, n_chunks * W_COLS], i32)
            nc.sync.dma_start(out=ctx_idx[:], in_=ctx_idx_d[:])
            nc.sync.dma_start(out=w_idx[:], in_=w_idx_d[:])

            acc = apool.tile([P, n_chunks], f32)       # per-chunk row losses
            ones = apool.tile([P, 1], f32)
            nc.vector.memset(ones[:], 1.0)

            for s in range(n_sc):
                ctx_g = gpool.tile([P, sc * CTX * DIM], f32, tag="ctx_g")
                w_g = gpool.tile([P, sc * W_COLS * DIM], f32, tag="w_g")

                nc.gpsimd.indirect_dma_start(
                    out=ctx_g[:],
                    out_offset=None,
                    in_=in_w_d[:],
                    in_offset=bass.IndirectOffsetOnAxis(
                        ap=ctx_idx[:, s * sc * CTX:(s + 1) * sc * CTX], axis=0
                    ),
                )
                nc.gpsimd.indirect_dma_start(
                    out=w_g[:],
                    out_offset=None,
                    in_=out_w_d[:],
                    in_offset=bass.IndirectOffsetOnAxis(
                        ap=w_idx[:, s * sc * W_COLS:(s + 1) * sc * W_COLS], axis=0
                    ),
                )

                for j in range(sc):
                    c = s * sc + j
                    # context sum over the CTX gathered rows:
                    # view [P, DIM(d), CTX(k)] with k innermost -> reduce X
                    ctx_view = ctx_g[:, j * CTX * DIM:(j + 1) * CTX * DIM].rearrange(
                        "p (k d) -> p d k", k=CTX
                    )
                    cs = wpool.tile([P, DIM], f32, tag="cs")
                    nc.vector.reduce_sum(out=cs[:], in_=ctx_view, axis=mybir.AxisListType.X)

                    # prod[p, t, d] = w_g[p, t, d] * cs[p, d]  for t in 0..10
                    w_view = w_g[:, j * W_COLS * DIM:(j + 1) * W_COLS * DIM]
                    prod = wpool.tile([P, W_COLS * DIM], f32, tag="prod")
                    nc.vector.tensor_mul(
                        out=prod[:],
                        in0=w_view,
                        in1=cs[:].rearrange("p (o d) -> p o d", o=1).broadcast_to(
                            [P, W_COLS, DIM]
                        ),
                    )
                    dots = wpool.tile([P, W_COLS], f32, tag="dots")
                    nc.vector.reduce_sum(
                        out=dots[:],
                        in_=prod[:].rearrange("p (t d) -> p t d", t=W_COLS),
                        axis=mybir.AxisListType.X,
                    )

                    # softplus(x/8) = ln(1 + exp(x/8)); center col gets -x/8
                    es = wpool.tile([P, W_COLS], f32, tag="es")
                    sp = wpool.tile([P, W_COLS], f32, tag="sp")
                    nc.scalar.activation(
                        out=es[:, 0:1], in_=dots[:, 0:1],
                        func=mybir.ActivationFunctionType.Exp, scale=-1.0 / CTX,
                    )
                    nc.scalar.activation(
                        out=es[:, 1:W_COLS], in_=dots[:, 1:W_COLS],
                        func=mybir.ActivationFunctionType.Exp, scale=1.0 / CTX,
                    )
                    nc.scalar.activation(
                        out=sp[:], in_=es[:],
                        func=mybir.ActivationFunctionType.Ln, bias=1.0,
                        accum_out=acc[:, c:c + 1],
                    )

            accv = apool.tile([P, 1], f32)
            nc.vector.reduce_sum(out=accv[:], in_=acc[:], axis=mybir.AxisListType.X)
            lsum = ppool.tile([1, 1], f32)
            nc.tensor.matmul(lsum[:], accv[:], ones[:], start=True, stop=True)
            res = apool.tile([1, 1], f32)
            nc.vector.tensor_copy(out=res[:], in_=lsum[:])
            nc.sync.dma_start(out=loss_d[:], in_=res[:])

    nc.finalize()
    return nc


def pack_indices(center, context, neg_context, n_chunks=N_CHUNKS):
    """Pack per-core indices into the SBUF layouts the kernel expects.

    ctx_idx [P, n_chunks*CTX]: [p, c*CTX + k] = context[c*128 + p, k]
    w_idx   [P, n_chunks*11]:  [p, c*11 + 0] = center row, +1.. = negatives
    """
    rows = n_chunks * P
    ctx_l, w_l = [], []
    for m in range(N_CORES):
        lo = m * rows
        ctx = np.ascontiguousarray(context[lo:lo + rows]).astype(np.int32)
        cen = np.ascontiguousarray(center[lo:lo + rows]).astype(np.int32)
        neg = np.ascontiguousarray(neg_context[lo:lo + rows]).astype(np.int32)
        ctx_p = ctx.reshape(n_chunks, P, CTX).transpose(1, 0, 2).reshape(P, n_chunks * CTX)
        w = np.concatenate([cen.reshape(rows, 1), neg.reshape(rows, K_NEG)], axis=1)
        w_p = w.reshape(n_chunks, P, W_COLS).transpose(1, 0, 2).reshape(P, n_chunks * W_COLS)
        ctx_l.append(np.ascontiguousarray(ctx_p))
        w_l.append(np.ascontiguousarray(w_p))
    return ctx_l, w_l


def kernel(center, context, neg_context, in_W, out_W):
    from concourse.bass_utils import run_bass_kernel_spmd

    if "nc" not in _CACHE:
        _CACHE["nc"] = build_bass()
    nc = _CACHE["nc"]

    ctx_l, w_l = pack_indices(np.asarray(center), np.asarray(context), np.asarray(neg_context))
    in_w = np.ascontiguousarray(np.asarray(in_W, dtype=np.float32))
    out_w = np.ascontiguousarray(np.asarray(out_W, dtype=np.float32))

    in_maps = [
        {"ctx_idx": ctx_l[m], "w_idx": w_l[m], "in_w": in_w, "out_w": out_w}
        for m in range(N_CORES)
    ]
    res = run_bass_kernel_spmd(nc, in_maps, core_ids=list(range(N_CORES)))
    total = sum(float(res.results[m]["loss"][0, 0]) for m in range(N_CORES))
    return np.float32(total / BATCH)


# revision 8
# speedup vs baseline: 1.1913x; 1.1913x over previous
"""CBOW negative-sampling loss kernel for Trainium2 (8 NeuronCores).

Problem (see reference):
    context_embeds = in_W[context].mean(axis=1)          # [B, D]
    true_embeds    = out_W[center.squeeze(1)]            # [B, D]
    pos_loss = softplus(-sum(context_embeds*true_embeds, -1)).mean()
    neg_embeds = out_W[neg_context]                      # [B, K, D]
    neg_loss = softplus(einsum('bkd,bd->bk', ...)).sum(-1).mean()
    out = pos_loss + neg_loss                            # scalar

Strategy: data-parallel over batch across 8 cores (2048 rows each);
embedding tables replicated per core.  Each core gathers its rows with
SWDGE indirect DMA (one 512B descriptor per embedding row), computes
dot products + softplus on DVE/ACT, and reduces to one partial-sum
scalar.  Host sums the 8 partials and divides by B.

Row layout per core: batch row b = chunk*128 + p lives on partition p,
chunk index c in the free dim (16 chunks of 128 rows).  Gathers are
issued per "super-chunk" of SC=4 chunks so DMA overlaps compute.

The walrus build in this container encodes at most ONE semaphore wait
per instruction ("Too many sync wait commands") and rejects the raw-ISA
InstTensorTensorReduce ("ISA wrong length"), so: waits are split onto
single-wait NoOps at Tile lowering time (PatchedTileContext below), and
dots use tensor_tensor + tensor_reduce instead.
"""

import numpy as np

VOCAB = 100000
DIM = 128
BATCH = 16384
CTX = 8
K_NEG = 10
N_CORES = 8
P = 128

B_CORE = BATCH // N_CORES          # 2048
N_CHUNKS = B_CORE // P             # 16
SC = 4                             # chunks per gather super-chunk
W_COLS = 1 + K_NEG                 # center + negatives share the out_W gather

_CACHE = {}


def _patched_tile_context():
    import concourse.mybir as mybir
    import concourse.tile as tile
    from concourse.vector_clock import ScopedClock

    class PatchedTileContext(tile.TileContext):
        """Split multi-wait sync_infos: this container's walrus codegen
        accepts only one semaphore wait (and update) per instruction."""

        def _add_instruction(self, inst):
            si = getattr(inst, "sync_info", None)
            if si is not None and len(si.on_wait) > 1:
                waits = list(si.on_wait)
                for w in waits[:-1]:
                    nop = mybir.InstNoOp(
                        name=f"I-{self.nc.next_id()}-waitsplit",
                        engine=inst.engine,
                        sync_info=mybir.SyncInfo(on_wait=[w], on_update=[]),
                        bass_nofuse=True,
                    )
                    super()._add_instruction(nop)
                inst.sync_info = mybir.SyncInfo(
                    on_wait=[waits[-1]], on_update=list(si.on_update)
                )
            super()._add_instruction(inst)

        def _drain_and_barrier(self, tick_clock, wait_clock):
            drain_inst = self.nc.sync.drain()
            wait_clock.add_sem_waits(
                drain_inst.ins, ScopedClock({None: tick_clock.global_clock})
            )
            si = drain_inst.ins.sync_info
            if si is not None and len(si.on_wait) > 1:
                waits = list(si.on_wait)
                ups = list(si.on_update)
                drain_inst.ins.sync_info = mybir.SyncInfo(
                    on_wait=waits[:1], on_update=[]
                )
                for i, w in enumerate(waits[1:]):
                    d2 = self.nc.sync.drain()
                    last = i == len(waits) - 2
                    d2.ins.sync_info = mybir.SyncInfo(
                        on_wait=[w], on_update=ups if last else []
                    )
            self.nc.all_engine_barrier()
            popped = self.nc._tile_sem_poison_stack.pop()
            assert popped is self._sem_poison
            self.nc.clear_and_free_semaphores(list(self.sems.allocated().values()))
            self.nc.all_engine_barrier()

    return PatchedTileContext


def build_bass(vocab=VOCAB, n_chunks=N_CHUNKS, sc=SC, gather_bufs=4):
    """Build the per-core Bass program.  Tables are gathered as bf16."""
    import concourse.bass as bass
    import concourse.mybir as mybir

    f32 = mybir.dt.float32
    bf16 = mybir.dt.bfloat16
    i32 = mybir.dt.int32
    n_sc = n_chunks // sc
    TileContext = _patched_tile_context()

    nc = bass.Bass()

    ctx_idx_d = nc.dram_tensor("ctx_idx", [P, n_chunks * CTX], i32, kind="ExternalInput")
    w_idx_d = nc.dram_tensor("w_idx", [P, n_chunks * W_COLS], i32, kind="ExternalInput")
    in_w_d = nc.dram_tensor("in_w", [vocab, DIM], bf16, kind="ExternalInput")
    out_w_d = nc.dram_tensor("out_w", [vocab, DIM], bf16, kind="ExternalInput")
    loss_d = nc.dram_tensor("loss", [1, 1], f32, kind="ExternalOutput")

    with TileContext(nc) as tc:
        with (
            nc.allow_low_precision(reason="bf16 dots are well within tolerance here"),
            tc.tile_pool(name="idx", bufs=1) as ipool,
            tc.tile_pool(name="gather", bufs=gather_bufs) as gpool,
            tc.tile_pool(name="work", bufs=2) as wpool,
            tc.tile_pool(name="accp", bufs=1) as apool,
            tc.tile_pool(name="psum", bufs=1, space="PSUM") as ppool,
        ):
            ctx_idx = ipool.tile([P, n_chunks * CTX], i32)
            w_idx = ipool.tile([P, n_chunks * W_COLS], i32)
            nc.sync.dma_start(out=ctx_idx[:], in_=ctx_idx_d[:])
            nc.sync.dma_start(out=w_idx[:], in_=w_idx_d[:])

            acc = apool.tile([P, n_chunks], f32)       # per-chunk row losses
            ones = apool.tile([P, 1], f32)
            nc.vector.memset(ones[:], 1.0)

            for s in range(n_sc):
                ctx_g = gpool.tile([P, sc * CTX * DIM], bf16, tag="ctx_g")
                w_g = gpool.tile([P, sc * W_COLS * DIM], bf16, tag="w_g")

                nc.gpsimd.indirect_dma_start(
                    out=ctx_g[:],
                    out_offset=None,
                    in_=in_w_d[:],
                    in_offset=bass.IndirectOffsetOnAxis(
                        ap=ctx_idx[:, s * sc * CTX:(s + 1) * sc * CTX], axis=0
                    ),
                )
                nc.gpsimd.indirect_dma_start(
                    out=w_g[:],
                    out_offset=None,
                    in_=out_w_d[:],
                    in_offset=bass.IndirectOffsetOnAxis(
                        ap=w_idx[:, s * sc * W_COLS:(s + 1) * sc * W_COLS], axis=0
                    ),
                )

                for j in range(sc):
                    c = s * sc + j
                    # context sum over the CTX gathered rows:
                    # view [P, DIM(d), CTX(k)] with k innermost -> reduce X
                    ctx_view = ctx_g[:, j * CTX * DIM:(j + 1) * CTX * DIM].rearrange(
                        "p (k d) -> p d k", k=CTX
                    )
                    cs = wpool.tile([P, DIM], bf16, tag="cs")
                    nc.vector.reduce_sum(out=cs[:], in_=ctx_view, axis=mybir.AxisListType.X)

                    # prod[p, t, d] = w_g[p, t, d] * cs[p, d]  for t in 0..10
                    w_view = w_g[:, j * W_COLS * DIM:(j + 1) * W_COLS * DIM]
                    prod = wpool.tile([P, W_COLS * DIM], bf16, tag="prod")
                    nc.vector.tensor_mul(
                        out=prod[:],
                        in0=w_view,
                        in1=cs[:].rearrange("p (o d) -> p o d", o=1).broadcast_to(
                            [P, W_COLS, DIM]
                        ),
                    )
                    dots = wpool.tile([P, W_COLS], f32, tag="dots")
                    nc.vector.reduce_sum(
                        out=dots[:],
                        in_=prod[:].rearrange("p (t d) -> p t d", t=W_COLS),
                        axis=mybir.AxisListType.X,
                    )

                    # softplus(x/8) = ln(1 + exp(x/8)); center col gets -x/8
                    es = wpool.tile([P, W_COLS], f32, tag="es")
                    sp = wpool.tile([P, W_COLS], f32, tag="sp")
                    nc.scalar.activation(
                        out=es[:, 0:1], in_=dots[:, 0:1],
                        func=mybir.ActivationFunctionType.Exp, scale=-1.0 / CTX,
                    )
                    nc.scalar.activation(
                        out=es[:, 1:W_COLS], in_=dots[:, 1:W_COLS],
                        func=mybir.ActivationFunctionType.Exp, scale=1.0 / CTX,
                    )
                    nc.scalar.activation(
                        out=sp[:], in_=es[:],
                        func=mybir.ActivationFunctionType.Ln, bias=1.0,
                        accum_out=acc[:, c:c + 1],
                    )

            accv = apool.tile([P, 1], f32)
            nc.vector.reduce_sum(out=accv[:], in_=acc[:], axis=mybir.AxisListType.X)
            lsum = ppool.tile([1, 1], f32)
            nc.tensor.matmul(lsum[:], accv[:], ones[:], start=True, stop=True)
            res = apool.tile([1, 1], f32)
            nc.vector.tensor_copy(out=res[:], in_=lsum[:])
            nc.sync.dma_start(out=loss_d[:], in_=res[:])

    nc.finalize()
    return nc


def pack_indices(center, context, neg_context, n_chunks=N_CHUNKS):
    """Pack per-core indices into the SBUF layouts the kernel expects.

    ctx_idx [P, n_chunks*CTX]: [p, c*CTX + k] = context[c*128 + p, k]
    w_idx   [P, n_chunks*11]:  [p, c*11 + 0] = center row, +1.. = negatives
    """
    rows = n_chunks * P
    ctx_l, w_l = [], []
    for m in range(N_CORES):
        lo = m * rows
        ctx = np.ascontiguousarray(context[lo:lo + rows]).astype(np.int32)
        cen = np.ascontiguousarray(center[lo:lo + rows]).astype(np.int32)
        neg = np.ascontiguousarray(neg_context[lo:lo + rows]).astype(np.int32)
        ctx_p = ctx.reshape(n_chunks, P, CTX).transpose(1, 0, 2).reshape(P, n_chunks * CTX)
        w = np.concatenate([cen.reshape(rows, 1), neg.reshape(rows, K_NEG)], axis=1)
        w_p = w.reshape(n_chunks, P, W_COLS).transpose(1, 0, 2).reshape(P, n_chunks * W_COLS)
        ctx_l.append(np.ascontiguousarray(ctx_p))
        w_l.append(np.ascontiguousarray(w_p))
    return ctx_l, w_l


def kernel(center, context, neg_context, in_W, out_W):
    from concourse.bass_utils import run_bass_kernel_spmd

    if "nc" not in _CACHE:
        _CACHE["nc"] = build_bass()
    nc = _CACHE["nc"]

    import ml_dtypes

    ctx_l, w_l = pack_indices(np.asarray(center), np.asarray(context), np.asarray(neg_context))
    in_w = np.ascontiguousarray(np.asarray(in_W, dtype=np.float32).astype(ml_dtypes.bfloat16))
    out_w = np.ascontiguousarray(np.asarray(out_W, dtype=np.float32).astype(ml_dtypes.bfloat16))

    in_maps = [
        {"ctx_idx": ctx_l[m], "w_idx": w_l[m], "in_w": in_w, "out_w": out_w}
        for m in range(N_CORES)
    ]
    res = run_bass_kernel_spmd(nc, in_maps, core_ids=list(range(N_CORES)))
    total = sum(float(res.results[m]["loss"][0, 0]) for m in range(N_CORES))
    return np.float32(total / BATCH)


# revision 14
# speedup vs baseline: 1.5374x; 1.2906x over previous
"""CBOW negative-sampling loss kernel for Trainium2 (8 NeuronCores).

Problem (see reference):
    context_embeds = in_W[context].mean(axis=1)          # [B, D]
    true_embeds    = out_W[center.squeeze(1)]            # [B, D]
    pos_loss = softplus(-sum(context_embeds*true_embeds, -1)).mean()
    neg_embeds = out_W[neg_context]                      # [B, K, D]
    neg_loss = softplus(einsum('bkd,bd->bk', ...)).sum(-1).mean()
    out = pos_loss + neg_loss                            # scalar

Strategy: data-parallel over batch across 8 cores (2048 rows each);
embedding tables replicated per core.  Each core gathers its rows with
SWDGE indirect DMA (one 512B descriptor per embedding row), computes
dot products + softplus on DVE/ACT, and reduces to one partial-sum
scalar.  Host sums the 8 partials and divides by B.

Row layout per core: batch row b = chunk*128 + p lives on partition p,
chunk index c in the free dim (16 chunks of 128 rows).  Gathers are
issued per "super-chunk" of SC=4 chunks so DMA overlaps compute.

The walrus build in this container encodes at most ONE semaphore wait
per instruction ("Too many sync wait commands") and rejects the raw-ISA
InstTensorTensorReduce ("ISA wrong length"), so: waits are split onto
single-wait NoOps at Tile lowering time (PatchedTileContext below), and
dots use tensor_tensor + tensor_reduce instead.
"""

import numpy as np

VOCAB = 100000
DIM = 128
BATCH = 16384
CTX = 8
K_NEG = 10
N_CORES = 8
P = 128

B_CORE = BATCH // N_CORES          # 2048
N_CHUNKS = B_CORE // P             # 16
SC = 4                             # chunks per gather super-chunk
W_COLS = 1 + K_NEG                 # center + negatives share the out_W gather

_CACHE = {}


def _patched_tile_context():
    import concourse.mybir as mybir
    import concourse.tile as tile
    from concourse.vector_clock import ScopedClock

    class PatchedTileContext(tile.TileContext):
        """Split multi-wait sync_infos: this container's walrus codegen
        accepts only one semaphore wait (and update) per instruction."""

        def _add_instruction(self, inst):
            si = getattr(inst, "sync_info", None)
            if si is not None and len(si.on_wait) > 1:
                waits = list(si.on_wait)
                for w in waits[:-1]:
                    nop = mybir.InstNoOp(
                        name=f"I-{self.nc.next_id()}-waitsplit",
                        engine=inst.engine,
                        sync_info=mybir.SyncInfo(on_wait=[w], on_update=[]),
                        bass_nofuse=True,
                    )
                    super()._add_instruction(nop)
                inst.sync_info = mybir.SyncInfo(
                    on_wait=[waits[-1]], on_update=list(si.on_update)
                )
            super()._add_instruction(inst)

        def _drain_and_barrier(self, tick_clock, wait_clock):
            drain_inst = self.nc.sync.drain()
            wait_clock.add_sem_waits(
                drain_inst.ins, ScopedClock({None: tick_clock.global_clock})
            )
            si = drain_inst.ins.sync_info
            if si is not None and len(si.on_wait) > 1:
                waits = list(si.on_wait)
                ups = list(si.on_update)
                drain_inst.ins.sync_info = mybir.SyncInfo(
                    on_wait=waits[:1], on_update=[]
                )
                for i, w in enumerate(waits[1:]):
                    d2 = self.nc.sync.drain()
                    last = i == len(waits) - 2
                    d2.ins.sync_info = mybir.SyncInfo(
                        on_wait=[w], on_update=ups if last else []
                    )
            self.nc.all_engine_barrier()
            popped = self.nc._tile_sem_poison_stack.pop()
            assert popped is self._sem_poison
            self.nc.clear_and_free_semaphores(list(self.sems.allocated().values()))
            self.nc.all_engine_barrier()

    return PatchedTileContext


def build_bass(vocab=VOCAB, n_chunks=N_CHUNKS, sc=SC, gather_bufs=4):
    """Build the per-core Bass program.  Tables are gathered as bf16."""
    import concourse.bass as bass
    import concourse.mybir as mybir

    f32 = mybir.dt.float32
    bf16 = mybir.dt.bfloat16
    i32 = mybir.dt.int32
    n_sc = n_chunks // sc
    TileContext = _patched_tile_context()

    nc = bass.Bass()

    ctx_idx_d = nc.dram_tensor("ctx_idx", [P, n_chunks * CTX], i32, kind="ExternalInput")
    w_idx_d = nc.dram_tensor("w_idx", [P, n_chunks * W_COLS], i32, kind="ExternalInput")
    in_w_d = nc.dram_tensor("in_w", [vocab, DIM], bf16, kind="ExternalInput")
    out_w_d = nc.dram_tensor("out_w", [vocab, DIM], bf16, kind="ExternalInput")
    loss_d = nc.dram_tensor("loss", [2, 1], f32, kind="ExternalOutput")

    with TileContext(nc) as tc:
        with (
            nc.allow_low_precision(reason="bf16 dots are well within tolerance here"),
            tc.tile_pool(name="idx", bufs=1) as ipool,
            tc.tile_pool(name="gather", bufs=gather_bufs) as gpool,
            tc.tile_pool(name="work", bufs=2) as wpool,
            tc.tile_pool(name="accp", bufs=1) as apool,
            tc.tile_pool(name="psum", bufs=1, space="PSUM") as ppool,
        ):
            ctx_idx = ipool.tile([P, n_chunks * CTX], i32)
            w_idx = ipool.tile([P, n_chunks * W_COLS], i32)
            nc.sync.dma_start(out=ctx_idx[:], in_=ctx_idx_d[:])
            nc.sync.dma_start(out=w_idx[:], in_=w_idx_d[:])

            acc = apool.tile([P, n_sc], f32)           # per-super-chunk row losses
            pos_acc = apool.tile([P, n_chunks], f32)   # raw pos dots per chunk
            ones = apool.tile([P, 1], f32)
            nc.vector.memset(ones[:], 1.0)

            for s in range(n_sc):
                ctx_g = gpool.tile([P, sc * CTX * DIM], bf16, tag="ctx_g")
                w_g = gpool.tile([P, sc * W_COLS * DIM], bf16, tag="w_g")

                nc.gpsimd.indirect_dma_start(
                    out=ctx_g[:],
                    out_offset=None,
                    in_=in_w_d[:],
                    in_offset=bass.IndirectOffsetOnAxis(
                        ap=ctx_idx[:, s * sc * CTX:(s + 1) * sc * CTX], axis=0
                    ),
                )
                nc.gpsimd.indirect_dma_start(
                    out=w_g[:],
                    out_offset=None,
                    in_=out_w_d[:],
                    in_offset=bass.IndirectOffsetOnAxis(
                        ap=w_idx[:, s * sc * W_COLS:(s + 1) * sc * W_COLS], axis=0
                    ),
                )

                # context sum over k (CTX gathered rows), whole super-chunk,
                # as a contiguous-inner tree of adds (DVE 2x bf16 mode; a
                # strided reduce-X runs at <1x and is ~3x slower)
                cv = ctx_g[:].rearrange("p (c k d) -> p c k d", c=sc, k=CTX)
                t1 = wpool.tile([P, sc * 4 * DIM], bf16, tag="t1")
                t1v = t1[:].rearrange("p (c k d) -> p c k d", c=sc, k=4)
                nc.vector.tensor_add(out=t1v, in0=cv[:, :, 0:4, :], in1=cv[:, :, 4:8, :])
                t2 = wpool.tile([P, sc * 2 * DIM], bf16, tag="t2")
                t2v = t2[:].rearrange("p (c k d) -> p c k d", c=sc, k=2)
                nc.vector.tensor_add(out=t2v, in0=t1v[:, :, 0:2, :], in1=t1v[:, :, 2:4, :])
                cs = wpool.tile([P, sc * DIM], bf16, tag="cs")
                csv = cs[:].rearrange("p (c o d) -> p c o d", c=sc, o=1)
                nc.vector.tensor_add(out=csv, in0=t2v[:, :, 0:1, :], in1=t2v[:, :, 1:2, :])

                # prod[p, c, t, d] = w_g[p, c, t, d] * cs[p, c, d]
                prod = wpool.tile([P, sc * W_COLS * DIM], bf16, tag="prod")
                nc.vector.tensor_mul(
                    out=prod[:],
                    in0=w_g[:],
                    in1=cs[:].rearrange("p (c o d) -> p c o d", c=sc, o=1).broadcast_to(
                        [P, sc, W_COLS, DIM]
                    ),
                )
                dots = wpool.tile([P, sc * W_COLS], f32, tag="dots")
                nc.vector.reduce_sum(
                    out=dots[:],
                    in_=prod[:].rearrange("p (c t d) -> p c t d", c=sc, t=W_COLS),
                    axis=mybir.AxisListType.X,
                )

                # softplus identity: softplus(-pos/8) = softplus(pos/8) - pos/8,
                # so apply softplus(x/8) to ALL 11 columns (contiguous ACT ops)
                # and subtract the pos dots at the end (host combines).
                es = wpool.tile([P, sc * W_COLS], f32, tag="es")
                sp = wpool.tile([P, sc * W_COLS], f32, tag="sp")
                nc.scalar.activation(
                    out=es[:], in_=dots[:],
                    func=mybir.ActivationFunctionType.Exp, scale=1.0 / CTX,
                )
                nc.scalar.activation(
                    out=sp[:], in_=es[:],
                    func=mybir.ActivationFunctionType.Ln, bias=1.0,
                    accum_out=acc[:, s:s + 1],
                )
                # stash the pos dots (t=0 column of each chunk) for correction
                nc.vector.tensor_copy(
                    out=pos_acc[:, s * sc:(s + 1) * sc],
                    in_=dots[:].rearrange("p (c t) -> p c t", t=W_COLS)[:, :, 0:1],
                )

            # partials [p, 0] = sum of softplus(x/8) terms, [p, 1] = sum of
            # raw pos dots; host computes (sum0 - sum1/8) / BATCH
            partials = apool.tile([P, 2], f32)
            nc.vector.reduce_sum(
                out=partials[:, 0:1], in_=acc[:], axis=mybir.AxisListType.X
            )
            nc.vector.reduce_sum(
                out=partials[:, 1:2], in_=pos_acc[:], axis=mybir.AxisListType.X
            )
            lsum = ppool.tile([2, 1], f32)
            nc.tensor.matmul(lsum[:], partials[:], ones[:], start=True, stop=True)
            res = apool.tile([2, 1], f32)
            nc.vector.tensor_copy(out=res[:], in_=lsum[:])
            nc.sync.dma_start(out=loss_d[:], in_=res[:])

    nc.finalize()
    return nc


def pack_indices(center, context, neg_context, n_chunks=N_CHUNKS):
    """Pack per-core indices into the SBUF layouts the kernel expects.

    ctx_idx [P, n_chunks*CTX]: [p, c*CTX + k] = context[c*128 + p, k]
    w_idx   [P, n_chunks*11]:  [p, c*11 + 0] = center row, +1.. = negatives
    """
    rows = n_chunks * P
    ctx_l, w_l = [], []
    for m in range(N_CORES):
        lo = m * rows
        ctx = np.ascontiguousarray(context[lo:lo + rows]).astype(np.int32)
        cen = np.ascontiguousarray(center[lo:lo + rows]).astype(np.int32)
        neg = np.ascontiguousarray(neg_context[lo:lo + rows]).astype(np.int32)
        ctx_p = ctx.reshape(n_chunks, P, CTX).transpose(1, 0, 2).reshape(P, n_chunks * CTX)
        w = np.concatenate([cen.reshape(rows, 1), neg.reshape(rows, K_NEG)], axis=1)
        w_p = w.reshape(n_chunks, P, W_COLS).transpose(1, 0, 2).reshape(P, n_chunks * W_COLS)
        ctx_l.append(np.ascontiguousarray(ctx_p))
        w_l.append(np.ascontiguousarray(w_p))
    return ctx_l, w_l


def kernel(center, context, neg_context, in_W, out_W):
    from concourse.bass_utils import run_bass_kernel_spmd

    if "nc" not in _CACHE:
        _CACHE["nc"] = build_bass()
    nc = _CACHE["nc"]

    import ml_dtypes

    ctx_l, w_l = pack_indices(np.asarray(center), np.asarray(context), np.asarray(neg_context))
    in_w = np.ascontiguousarray(np.asarray(in_W, dtype=np.float32).astype(ml_dtypes.bfloat16))
    out_w = np.ascontiguousarray(np.asarray(out_W, dtype=np.float32).astype(ml_dtypes.bfloat16))

    in_maps = [
        {"ctx_idx": ctx_l[m], "w_idx": w_l[m], "in_w": in_w, "out_w": out_w}
        for m in range(N_CORES)
    ]
    res = run_bass_kernel_spmd(nc, in_maps, core_ids=list(range(N_CORES)))
    total = sum(
        float(res.results[m]["loss"][0, 0]) - float(res.results[m]["loss"][1, 0]) / CTX
        for m in range(N_CORES)
    )
    return np.float32(total / BATCH)


# revision 17
# speedup vs baseline: 1.6882x; 1.0981x over previous
"""CBOW negative-sampling loss kernel for Trainium2 (8 NeuronCores).

Problem (see reference):
    context_embeds = in_W[context].mean(axis=1)          # [B, D]
    true_embeds    = out_W[center.squeeze(1)]            # [B, D]
    pos_loss = softplus(-sum(context_embeds*true_embeds, -1)).mean()
    neg_embeds = out_W[neg_context]                      # [B, K, D]
    neg_loss = softplus(einsum('bkd,bd->bk', ...)).sum(-1).mean()
    out = pos_loss + neg_loss                            # scalar

Strategy: data-parallel over batch across 8 cores (2048 rows each);
embedding tables replicated per core.  Each core gathers its rows with
SWDGE indirect DMA (one 512B descriptor per embedding row), computes
dot products + softplus on DVE/ACT, and reduces to one partial-sum
scalar.  Host sums the 8 partials and divides by B.

Row layout per core: batch row b = chunk*128 + p lives on partition p,
chunk index c in the free dim (16 chunks of 128 rows).  Gathers are
issued per "super-chunk" of SC=4 chunks so DMA overlaps compute.

The walrus build in this container encodes at most ONE semaphore wait
per instruction ("Too many sync wait commands") and rejects the raw-ISA
InstTensorTensorReduce ("ISA wrong length"), so: waits are split onto
single-wait NoOps at Tile lowering time (PatchedTileContext below), and
dots use tensor_tensor + tensor_reduce instead.
"""

import numpy as np

VOCAB = 100000
DIM = 128
BATCH = 16384
CTX = 8
K_NEG = 10
N_CORES = 8
P = 128

B_CORE = BATCH // N_CORES          # 2048
N_CHUNKS = B_CORE // P             # 16
SC = 4                             # chunks per gather super-chunk
W_COLS = 1 + K_NEG                 # center + negatives share the out_W gather

_CACHE = {}


def _patched_tile_context():
    import concourse.mybir as mybir
    import concourse.tile as tile
    from concourse.vector_clock import ScopedClock

    class PatchedTileContext(tile.TileContext):
        """Split multi-wait sync_infos: this container's walrus codegen
        accepts only one semaphore wait (and update) per instruction."""

        def _add_instruction(self, inst):
            si = getattr(inst, "sync_info", None)
            if si is not None and len(si.on_wait) > 1:
                waits = list(si.on_wait)
                for w in waits[:-1]:
                    nop = mybir.InstNoOp(
                        name=f"I-{self.nc.next_id()}-waitsplit",
                        engine=inst.engine,
                        sync_info=mybir.SyncInfo(on_wait=[w], on_update=[]),
                        bass_nofuse=True,
                    )
                    super()._add_instruction(nop)
                inst.sync_info = mybir.SyncInfo(
                    on_wait=[waits[-1]], on_update=list(si.on_update)
                )
            super()._add_instruction(inst)

        def _drain_and_barrier(self, tick_clock, wait_clock):
            drain_inst = self.nc.sync.drain()
            wait_clock.add_sem_waits(
                drain_inst.ins, ScopedClock({None: tick_clock.global_clock})
            )
            si = drain_inst.ins.sync_info
            if si is not None and len(si.on_wait) > 1:
                waits = list(si.on_wait)
                ups = list(si.on_update)
                drain_inst.ins.sync_info = mybir.SyncInfo(
                    on_wait=waits[:1], on_update=[]
                )
                for i, w in enumerate(waits[1:]):
                    d2 = self.nc.sync.drain()
                    last = i == len(waits) - 2
                    d2.ins.sync_info = mybir.SyncInfo(
                        on_wait=[w], on_update=ups if last else []
                    )
            self.nc.all_engine_barrier()
            popped = self.nc._tile_sem_poison_stack.pop()
            assert popped is self._sem_poison
            self.nc.clear_and_free_semaphores(list(self.sems.allocated().values()))
            self.nc.all_engine_barrier()

    return PatchedTileContext


def build_bass(vocab=VOCAB, n_chunks=N_CHUNKS, sc=SC, gather_bufs=1):
    """Build the per-core Bass program.  Tables are gathered as bf16."""
    import concourse.bass as bass
    import concourse.mybir as mybir

    f32 = mybir.dt.float32
    bf16 = mybir.dt.bfloat16
    i32 = mybir.dt.int32
    n_sc = n_chunks // sc
    TileContext = _patched_tile_context()

    nc = bass.Bass()

    ctx_idx_d = nc.dram_tensor("ctx_idx", [P, n_chunks * CTX], i32, kind="ExternalInput")
    w_idx_d = nc.dram_tensor("w_idx", [P, n_chunks * W_COLS], i32, kind="ExternalInput")
    in_w_d = nc.dram_tensor("in_w", [vocab, DIM], bf16, kind="ExternalInput")
    out_w_d = nc.dram_tensor("out_w", [vocab, DIM], bf16, kind="ExternalInput")
    loss_d = nc.dram_tensor("loss", [2, 1], f32, kind="ExternalOutput")

    with TileContext(nc) as tc:
        with (
            nc.allow_low_precision(reason="bf16 dots are well within tolerance here"),
            tc.tile_pool(name="idx", bufs=1) as ipool,
            tc.tile_pool(name="gather", bufs=gather_bufs) as gpool,
            tc.tile_pool(name="work", bufs=2) as wpool,
            tc.tile_pool(name="accp", bufs=1) as apool,
            tc.tile_pool(name="psum", bufs=1, space="PSUM") as ppool,
        ):
            ctx_idx = ipool.tile([P, n_chunks * CTX], i32)
            w_idx = ipool.tile([P, n_chunks * W_COLS], i32)
            nc.sync.dma_start(out=ctx_idx[:], in_=ctx_idx_d[:])
            nc.sync.dma_start(out=w_idx[:], in_=w_idx_d[:])

            acc = apool.tile([P, n_sc], f32)           # per-super-chunk row losses
            pos_acc = apool.tile([P, n_chunks], f32)   # raw pos dots per chunk
            ones = apool.tile([P, 1], f32)
            nc.vector.memset(ones[:], 1.0)

            # issue ALL gathers first so the Pool engine streams descriptors
            # back-to-back and the SDMA queue never starves
            sc_tiles = []
            for s in range(n_sc):
                ctx_g = gpool.tile([P, sc * CTX * DIM], bf16, tag=f"ctx_g{s}")
                w_g = gpool.tile([P, sc * W_COLS * DIM], bf16, tag=f"w_g{s}")
                nc.gpsimd.indirect_dma_start(
                    out=ctx_g[:],
                    out_offset=None,
                    in_=in_w_d[:],
                    in_offset=bass.IndirectOffsetOnAxis(
                        ap=ctx_idx[:, s * sc * CTX:(s + 1) * sc * CTX], axis=0
                    ),
                )
                nc.gpsimd.indirect_dma_start(
                    out=w_g[:],
                    out_offset=None,
                    in_=out_w_d[:],
                    in_offset=bass.IndirectOffsetOnAxis(
                        ap=w_idx[:, s * sc * W_COLS:(s + 1) * sc * W_COLS], axis=0
                    ),
                )
                sc_tiles.append((ctx_g, w_g))

            for s in range(n_sc):
                ctx_g, w_g = sc_tiles[s]
                # context sum over k (CTX gathered rows), whole super-chunk,
                # as a contiguous-inner tree of adds (DVE 2x bf16 mode; a
                # strided reduce-X runs at <1x and is ~3x slower)
                cv = ctx_g[:].rearrange("p (c k d) -> p c k d", c=sc, k=CTX)
                t1 = wpool.tile([P, sc * 4 * DIM], bf16, tag="t1")
                t1v = t1[:].rearrange("p (c k d) -> p c k d", c=sc, k=4)
                nc.vector.tensor_add(out=t1v, in0=cv[:, :, 0:4, :], in1=cv[:, :, 4:8, :])
                t2 = wpool.tile([P, sc * 2 * DIM], bf16, tag="t2")
                t2v = t2[:].rearrange("p (c k d) -> p c k d", c=sc, k=2)
                nc.vector.tensor_add(out=t2v, in0=t1v[:, :, 0:2, :], in1=t1v[:, :, 2:4, :])
                cs = wpool.tile([P, sc * DIM], bf16, tag="cs")
                csv = cs[:].rearrange("p (c o d) -> p c o d", c=sc, o=1)
                nc.vector.tensor_add(out=csv, in0=t2v[:, :, 0:1, :], in1=t2v[:, :, 1:2, :])

                # prod[p, c, t, d] = w_g[p, c, t, d] * cs[p, c, d]
                prod = wpool.tile([P, sc * W_COLS * DIM], bf16, tag="prod")
                nc.vector.tensor_mul(
                    out=prod[:],
                    in0=w_g[:],
                    in1=cs[:].rearrange("p (c o d) -> p c o d", c=sc, o=1).broadcast_to(
                        [P, sc, W_COLS, DIM]
                    ),
                )
                # fold d 128 -> 32 with adds (2x mode) before the 1x reduce
                pv = prod[:].rearrange("p (c t h d) -> p c t h d", c=sc, t=W_COLS, h=2)
                f1 = wpool.tile([P, sc * W_COLS * 64], bf16, tag="f1")
                f1v = f1[:].rearrange("p (c t h d) -> p c t h d", c=sc, t=W_COLS, h=2)
                nc.vector.tensor_add(
                    out=f1[:].rearrange("p (c t d) -> p c t d", c=sc, t=W_COLS),
                    in0=pv[:, :, :, 0, :], in1=pv[:, :, :, 1, :],
                )
                f2 = wpool.tile([P, sc * W_COLS * 32], bf16, tag="f2")
                nc.vector.tensor_add(
                    out=f2[:].rearrange("p (c t d) -> p c t d", c=sc, t=W_COLS),
                    in0=f1v[:, :, :, 0, :], in1=f1v[:, :, :, 1, :],
                )
                dots = wpool.tile([P, sc * W_COLS], f32, tag="dots")
                nc.vector.reduce_sum(
                    out=dots[:],
                    in_=f2[:].rearrange("p (c t d) -> p c t d", c=sc, t=W_COLS),
                    axis=mybir.AxisListType.X,
                )

                # softplus identity: softplus(-pos/8) = softplus(pos/8) - pos/8,
                # so apply softplus(x/8) to ALL 11 columns (contiguous ACT ops)
                # and subtract the pos dots at the end (host combines).
                es = wpool.tile([P, sc * W_COLS], f32, tag="es")
                sp = wpool.tile([P, sc * W_COLS], f32, tag="sp")
                nc.scalar.activation(
                    out=es[:], in_=dots[:],
                    func=mybir.ActivationFunctionType.Exp, scale=1.0 / CTX,
                )
                nc.scalar.activation(
                    out=sp[:], in_=es[:],
                    func=mybir.ActivationFunctionType.Ln, bias=1.0,
                    accum_out=acc[:, s:s + 1],
                )
                # stash the pos dots (t=0 column of each chunk) for correction
                nc.vector.tensor_copy(
                    out=pos_acc[:, s * sc:(s + 1) * sc],
                    in_=dots[:].rearrange("p (c t) -> p c t", t=W_COLS)[:, :, 0:1],
                )

            # partials [p, 0] = sum of softplus(x/8) terms, [p, 1] = sum of
            # raw pos dots; host computes (sum0 - sum1/8) / BATCH
            partials = apool.tile([P, 2], f32)
            nc.vector.reduce_sum(
                out=partials[:, 0:1], in_=acc[:], axis=mybir.AxisListType.X
            )
            nc.vector.reduce_sum(
                out=partials[:, 1:2], in_=pos_acc[:], axis=mybir.AxisListType.X
            )
            lsum = ppool.tile([2, 1], f32)
            nc.tensor.matmul(lsum[:], partials[:], ones[:], start=True, stop=True)
            res = apool.tile([2, 1], f32)
            nc.vector.tensor_copy(out=res[:], in_=lsum[:])
            nc.sync.dma_start(out=loss_d[:], in_=res[:])

    nc.finalize()
    return nc


def pack_indices(center, context, neg_context, n_chunks=N_CHUNKS):
    """Pack per-core indices into the SBUF layouts the kernel expects.

    ctx_idx [P, n_chunks*CTX]: [p, c*CTX + k] = context[c*128 + p, k]
    w_idx   [P, n_chunks*11]:  [p, c*11 + 0] = center row, +1.. = negatives
    """
    rows = n_chunks * P
    ctx_l, w_l = [], []
    for m in range(N_CORES):
        lo = m * rows
        ctx = np.ascontiguousarray(context[lo:lo + rows]).astype(np.int32)
        cen = np.ascontiguousarray(center[lo:lo + rows]).astype(np.int32)
        neg = np.ascontiguousarray(neg_context[lo:lo + rows]).astype(np.int32)
        ctx_p = ctx.reshape(n_chunks, P, CTX).transpose(1, 0, 2).reshape(P, n_chunks * CTX)
        w = np.concatenate([cen.reshape(rows, 1), neg.reshape(rows, K_NEG)], axis=1)
        w_p = w.reshape(n_chunks, P, W_COLS).transpose(1, 0, 2).reshape(P, n_chunks * W_COLS)
        ctx_l.append(np.ascontiguousarray(ctx_p))
        w_l.append(np.ascontiguousarray(w_p))
    return ctx_l, w_l


def kernel(center, context, neg_context, in_W, out_W):
    from concourse.bass_utils import run_bass_kernel_spmd

    if "nc" not in _CACHE:
        _CACHE["nc"] = build_bass()
    nc = _CACHE["nc"]

    import ml_dtypes

    ctx_l, w_l = pack_indices(np.asarray(center), np.asarray(context), np.asarray(neg_context))
    in_w = np.ascontiguousarray(np.asarray(in_W, dtype=np.float32).astype(ml_dtypes.bfloat16))
    out_w = np.ascontiguousarray(np.asarray(out_W, dtype=np.float32).astype(ml_dtypes.bfloat16))

    in_maps = [
        {"ctx_idx": ctx_l[m], "w_idx": w_l[m], "in_w": in_w, "out_w": out_w}
        for m in range(N_CORES)
    ]
    res = run_bass_kernel_spmd(nc, in_maps, core_ids=list(range(N_CORES)))
    total = sum(
        float(res.results[m]["loss"][0, 0]) - float(res.results[m]["loss"][1, 0]) / CTX
        for m in range(N_CORES)
    )
    return np.float32(total / BATCH)
